# revision 23
# baseline (speedup 1.0000x reference)
"""Trainium2 Bass kernel for per-variable-MLP GNN message passing.

Model (reference):
    adj  = ones(D,D) - eye(D)                       # var t cannot see itself
    h0   = leaky_relu(einsum('tij,bj->bti', w0*adjmask, x) + b0)
    h1   = leaky_relu(einsum('tij,btj->bti', w1, h0) + b1)
    out  = einsum('tij,btj->bti', w2, h1) + b2      # (B, D, O)

Sharding: the variable axis t (128) is split across 8 cores (16 vars each);
each core sees the full batch. Within a core, variables are processed in
pairs: a pair's two (64 x K) weight matrices are stacked/block-diagonalized
to fill the 128-wide tensor-engine array; activations live transposed
(feature-on-partition, batch-on-free).

Throughput structure:
- The PSUM->SBUF bias+leaky epilogues are the port-bound limiter (1
  elem/lane/cycle on ScalarE or VectorE); they are split between ScalarE
  (fused Prelu-with-bias) and VectorE (+GpSimd for the SBUF-side half).
- Emission is software-pipelined by batch chunk (L0 of chunk k, L1 of
  chunk k-1, L2 of chunk k-2) so the in-order TensorE queue never parks
  behind an epilogue, keeping the PE dense and HAM-warm.
- L1 runs as one 128x128 block-diagonal matmul per pair (L1_MODE="2x2"
  selects an alternative 2x2 tile_position packing; measured slower on
  HW because only diagonal quadrant pairs overlap).
- L2 packs 4 pairs per PSUM tile via column tile_position; one VectorE
  bias-add evacuates 4 pairs; the host picks the 16 valid rows out of
  the wide (128, B) outputs.

Matmuls run in fp16 (1 cycle/row on the PE, fp32 accumulate in PSUM).
The kernel emits a (32, 8192) transposed output per core; the host
reassembles (8192, 128, 2).
"""

import numpy as np

import concourse.bass as bass
import concourse.mybir as mybir
import concourse.tile as tile
from concourse import bacc, bass_utils

F32 = mybir.dt.float32
DT = mybir.dt.float16
NPDT = np.float16

B = 8192  # batch
D = 128  # num variables (t)
H = 64  # hidden
O = 2  # output dim per variable
NCORES = 8
TPC = D // NCORES  # vars per core = 16
NPAIR = TPC // 2  # 8
CH = 512  # batch chunk (psum bank = 512 fp32)
NCHUNK = B // CH  # 16
ALPHA = 0.01  # leaky_relu slope

L1_MODE = "bd"  # "bd" = block-diag single MM per pair; "2x2" = tile_position

Prelu = mybir.ActivationFunctionType.Prelu
MULT = mybir.AluOpType.mult
MAX = mybir.AluOpType.max


def _epilogue_kind(c, p, layer):
    """Work split for the L0/L1 epilogues: 'act' (ScalarE, fused) or
    'dve' (VectorE bias-add + leaky on VectorE/GpSimd)."""
    idx = (c * NPAIR + p) * 2 + layer
    return "dve" if idx % 7 in (0, 4) else "act"


def _build_program():
    nc = bacc.Bacc(trn_type="TRN2")

    xt = nc.dram_tensor("xt", (D, B), DT, kind="ExternalInput")
    w0t = nc.dram_tensor("w0t", (D, NPAIR * 128), DT, kind="ExternalInput")
    w1bd = nc.dram_tensor("w1bd", (128, NPAIR * 128), DT, kind="ExternalInput")
    w2bd = nc.dram_tensor("w2bd", (128, NPAIR * 8), DT, kind="ExternalInput")
    b0c = nc.dram_tensor("b0c", (128, NPAIR), F32, kind="ExternalInput")
    b1c = nc.dram_tensor("b1c", (128, NPAIR), F32, kind="ExternalInput")
    # b2c: per-partition bias; partition 32q+r = pair q row r,
    # partition 32q+4+r = pair 4+q row r
    b2c = nc.dram_tensor("b2c", (128, 1), F32, kind="ExternalInput")
    # wide output: full 128-partition evac tiles; host picks valid rows
    otA = nc.dram_tensor("otA", (128, B), F32, kind="ExternalOutput")

    with tile.TileContext(nc) as tc:
        with (
            tc.tile_pool(name="wp", bufs=1) as wp,
            tc.tile_pool(name="hp", bufs=20) as hp,
            tc.tile_pool(name="op", bufs=4) as op,
            tc.tile_pool(name="z0p", bufs=3, space="PSUM") as z0p,
            tc.tile_pool(name="z1p", bufs=3, space="PSUM") as z1p,
            tc.tile_pool(name="z2p", bufs=2, space="PSUM") as z2p,
        ):
            xs = wp.tile([D, B], DT)
            xs_loaded = [False] * NCHUNK
            xs_loaded[0] = True
            w0s = wp.tile([D, NPAIR * 128], DT)
            w1s = wp.tile([128, NPAIR * 128], DT)
            w2s = wp.tile([128, NPAIR * 8], DT)
            b0s = wp.tile([128, NPAIR], F32)
            b1s = wp.tile([128, NPAIR], F32)
            b2s = wp.tile([128, 1], F32)
            nc.sync.dma_start(xs[:, 0:CH], xt[:, 0:CH])
            nc.sync.dma_start(w0s[:], w0t[:])
            nc.sync.dma_start(b0s[:], b0c[:])
            nc.sync.dma_start(w1s[:], w1bd[:])
            nc.sync.dma_start(b1s[:], b1c[:])
            nc.sync.dma_start(w2s[:], w2bd[:])
            nc.sync.dma_start(b2s[:], b2c[:])

            def leaky_epilogue(dst, z, bias_col, kind):
                """dst (fp16 SBUF) = leaky_relu(z + bias), z in PSUM."""
                if kind == "act":
                    nc.scalar.activation(
                        dst[:], z[:], Prelu, bias=bias_col, scale=1.0, alpha=ALPHA
                    )
                else:
                    y = hp.tile([128, CH], DT, tag="y", bufs=8)
                    nc.vector.tensor_scalar_add(y[:], z[:], bias_col)
                    eng = nc.gpsimd if kind == "dve_gps" else nc.vector
                    eng.scalar_tensor_tensor(dst[:], y[:], ALPHA, y[:], MULT, MAX)

            # PE warmup: dummy matmuls with no input-DMA dependency so the
            # HAM clock-gate reaches 8/8 while the input DMAs run.
            warm = wp.tile([128, CH], DT, name="warm")
            nc.gpsimd.memset(warm[:], 0.0)
            wps = z1p.tile([128, CH], F32, name="warmps", tag="z1")
            for _ in range(8):
                nc.tensor.matmul(wps[:], warm[:, 0:128], warm[:], start=True,
                                 stop=True)

            # per-chunk state carried across pipeline stages
            h0_tiles = [None] * NCHUNK  # list of 8 tiles per chunk
            h1_tiles = [None] * NCHUNK

            for k in range(NCHUNK + 2):
                # ---- stage A: L0 for chunk k ----
                if k < NCHUNK:
                    c = k
                    cs = bass.ts(c, CH)
                    if not xs_loaded[c]:
                        nc.sync.dma_start(xs[:, cs], xt[:, cs])
                        xs_loaded[c] = True
                    tiles = []
                    for p in range(NPAIR):
                        z0 = z0p.tile([128, CH], F32, tag="z0")
                        nc.tensor.matmul(
                            z0[:], w0s[:, bass.ts(p, 128)], xs[:, cs],
                            start=True, stop=True,
                        )
                        h0 = hp.tile([128, CH], DT, tag="h0", name=f"h0_{c}_{p}",
                                     bufs=20)
                        leaky_epilogue(h0, z0, b0s[:, p : p + 1],
                                       _epilogue_kind(c, p, 0))
                        tiles.append(h0)
                    h0_tiles[c] = tiles

                # ---- stage B: L1 for chunk k-1 (2x2 tile_position) ----
                if 1 <= k <= NCHUNK:
                    c = k - 1
                    tiles = []
                    z1s = []
                    if L1_MODE == "2x2":
                        for g in range(NPAIR // 2):
                            pA, pB = 2 * g, 2 * g + 1
                            h0A, h0B = h0_tiles[c][pA], h0_tiles[c][pB]
                            z1A = z1p.tile([128, CH], F32, tag="z1",
                                           name=f"z1A_{c}_{g}")
                            z1B = z1p.tile([128, CH], F32, tag="z1",
                                           name=f"z1B_{c}_{g}")
                            nc.tensor.matmul(
                                z1A[0:H, :], w1s[0:H, 128 * pA : 128 * pA + H],
                                h0A[0:H, :], start=True, stop=True,
                                tile_position=(0, 0),
                            )
                            nc.tensor.matmul(
                                z1A[H:128, :],
                                w1s[H:128, 128 * pA + H : 128 * pA + 128],
                                h0A[H:128, :], start=True, stop=True,
                                tile_position=(64, 64),
                            )
                            # pair B: outputs land var-swapped (odd var at 0:64)
                            nc.tensor.matmul(
                                z1B[H:128, :],
                                w1s[0:H, 128 * pB + H : 128 * pB + 128],
                                h0B[0:H, :], start=True, stop=True,
                                tile_position=(0, 64),
                            )
                            nc.tensor.matmul(
                                z1B[0:H, :], w1s[H:128, 128 * pB : 128 * pB + H],
                                h0B[H:128, :], start=True, stop=True,
                                tile_position=(64, 0),
                            )
                            z1s.extend([z1A, z1B])
                    else:
                        for p in range(NPAIR):
                            z1 = z1p.tile([128, CH], F32, tag="z1",
                                          name=f"z1_{c}_{p}")
                            nc.tensor.matmul(
                                z1[:], w1s[:, bass.ts(p, 128)],
                                h0_tiles[c][p][:], start=True, stop=True,
                            )
                            z1s.append(z1)
                    for p in range(NPAIR):
                        h1 = hp.tile([128, CH], DT, tag="h1", name=f"h1_{c}_{p}",
                                     bufs=20)
                        leaky_epilogue(h1, z1s[p], b1s[:, p : p + 1],
                                       _epilogue_kind(c, p, 1))
                        tiles.append(h1)
                    h1_tiles[c] = tiles

                # ---- stage C: L2 + evac + store for chunk k-2 ----
                if 2 <= k:
                    c = k - 2
                    cs = bass.ts(c, CH)
                    z2 = z2p.tile([128, CH], F32, name=f"z2_{c}", tag="z2")
                    for q in range(4):
                        # pair 4+q first: M=8 zero-padded weights write zeros
                        # into rows 32q..32q+4 (start=True clears), data into
                        # rows 32q+4..32q+8
                        pB = 4 + q
                        nc.tensor.matmul(
                            z2[32 * q : 32 * q + 8, :],
                            w2s[:, 8 * pB : 8 * pB + 8],
                            h1_tiles[c][pB][:],
                            start=True, stop=False,
                            tile_position=(0, 32 * q),
                        )
                        # pair q accumulates into rows 32q..32q+4
                        pA = q
                        nc.tensor.matmul(
                            z2[32 * q : 32 * q + 4, :],
                            w2s[:, 8 * pA : 8 * pA + 4],
                            h1_tiles[c][pA][:],
                            start=False, stop=True,
                            tile_position=(0, 32 * q),
                        )
                    ob = op.tile([128, CH], F32, tag="ob")
                    nc.vector.tensor_scalar_add(ob[:], z2[:], b2s[:, 0:1])
                    nc.sync.dma_start(otA[:, cs], ob[:])
                    h1_tiles[c] = None

    nc.finalize()
    return nc


_prog = None


def _get_program():
    global _prog
    if _prog is None:
        _prog = _build_program()
    return _prog


def _shard_inputs(x, w0, w1, w2, b0, b1, b2):
    """Host-side relayout + t-sharding. Returns list of 8 in_maps."""
    x = np.asarray(x, np.float32)
    w0 = np.array(w0, np.float32)  # copy: we zero the adjacency diagonal
    w1 = np.asarray(w1, np.float32)
    w2 = np.asarray(w2, np.float32)
    b0 = np.asarray(b0, np.float32)
    b1 = np.asarray(b1, np.float32)
    b2 = np.asarray(b2, np.float32)

    # adjacency mask: variable t cannot see itself -> w0[t, :, t] = 0
    ar = np.arange(D)
    w0[ar, :, ar] = 0.0

    xt = np.ascontiguousarray(x.T).astype(NPDT)  # (128, 8192)

    in_maps = []
    for c in range(NCORES):
        ts_ = slice(c * TPC, (c + 1) * TPC)
        w0c, w1c, w2c = w0[ts_], w1[ts_], w2[ts_]
        b0cc, b1cc, b2cc = b0[ts_], b1[ts_], b2[ts_]

        # w0t: (128 j, pair*128 + [ta's 64 i | tb's 64 i])
        w0T = w0c.transpose(0, 2, 1)  # (16, 128 j, 64 i)
        w0t_ = np.ascontiguousarray(
            w0T.reshape(NPAIR, 2, D, H).transpose(2, 0, 1, 3).reshape(D, NPAIR * 128)
        ).astype(NPDT)

        # w1bd: per-pair 128x128 blocks, K rows = h0 pair stack.
        # Even pairs: diagonal blocks; odd pairs: anti-diagonal (outputs come
        # out var-swapped from the 2x2 tile_position packing).
        bd1 = np.zeros((NPAIR, 128, 128), np.float32)
        for p in range(NPAIR):
            te, to = w1c[2 * p].T, w1c[2 * p + 1].T  # (in, out) each (64,64)
            if L1_MODE == "bd" or p % 2 == 0:
                bd1[p, 0:H, 0:H] = te
                bd1[p, H:128, H:128] = to
            else:
                bd1[p, 0:H, H:128] = te
                bd1[p, H:128, 0:H] = to
        w1bd_ = np.ascontiguousarray(
            bd1.transpose(1, 0, 2).reshape(128, NPAIR * 128)
        ).astype(NPDT)

        # b1c: per-partition bias for h1 tiles; odd pairs are var-swapped
        b1q = b1cc.reshape(NPAIR, 2, H)
        b1sw = b1q.copy()
        if L1_MODE == "2x2":
            b1sw[1::2] = b1q[1::2, ::-1]
        b1c_ = np.ascontiguousarray(b1sw.reshape(NPAIR, 128).T).astype(np.float32)

        # w2bd: (128 K, pair*4 + [ta o0, ta o1, tb o0, tb o1]); K rows follow
        # the h1 layout (odd pairs var-swapped), output cols stay canonical.
        bd2 = np.zeros((NPAIR, 128, 8), np.float32)
        for p in range(NPAIR):
            te, to = w2c[2 * p].T, w2c[2 * p + 1].T  # (64, 2) each
            off = 0 if p < 4 else 4  # B pairs sit 4 cols right (zero-padded)
            bd2[p, 0:H, off : off + 2] = te
            bd2[p, H:128, off + 2 : off + 4] = to
        w2bd_ = np.ascontiguousarray(
            bd2.transpose(1, 0, 2).reshape(128, NPAIR * 8)
        ).astype(NPDT)

        b0c_ = np.ascontiguousarray(b0cc.reshape(NPAIR, 128).T).astype(np.float32)
        # b2 bias layout for col-packed L2 psums: partition 32q+r of psum
        # `half` holds pair (4*half+q) row r (r = 2*two + o), canonical order
        b2q = b2cc.reshape(NPAIR, 4)
        b2c_ = np.zeros((128, 1), np.float32)
        for q in range(4):
            b2c_[32 * q : 32 * q + 4, 0] = b2q[q]
            b2c_[32 * q + 4 : 32 * q + 8, 0] = b2q[4 + q]

        in_maps.append(
            {
                "xt": xt,
                "w0t": w0t_,
                "w1bd": w1bd_,
                "w2bd": w2bd_,
                "b0c": b0c_,
                "b1c": b1c_,
                "b2c": b2c_,
            }
        )
    return in_maps


def _unshard_outputs(results):
    out = np.empty((B, D, O), np.float32)
    sel = np.array(
        [
            32 * p + r if p < 4 else 32 * (p - 4) + 4 + r
            for p in range(NPAIR)
            for r in range(4)
        ]
    )
    for c in range(NCORES):
        ot = results[c]["otA"][sel]  # (32, 8192): row = 4p + 2*two + o
        blk = ot.reshape(NPAIR, 2, O, B).transpose(3, 0, 1, 2).reshape(B, TPC, O)
        out[:, c * TPC : (c + 1) * TPC, :] = blk
    return out


def kernel(x, w0, w1, w2, b0, b1, b2):
    nc = _get_program()
    in_maps = _shard_inputs(x, w0, w1, w2, b0, b1, b2)
    res = bass_utils.run_bass_kernel_spmd(nc, in_maps, core_ids=list(range(NCORES)))
    return _unshard_outputs(res.results)


# revision 25
# speedup vs baseline: 1.0034x; 1.0034x over previous
"""Trainium2 Bass kernel for per-variable-MLP GNN message passing.

Model (reference):
    adj  = ones(D,D) - eye(D)                       # var t cannot see itself
    h0   = leaky_relu(einsum('tij,bj->bti', w0*adjmask, x) + b0)
    h1   = leaky_relu(einsum('tij,btj->bti', w1, h0) + b1)
    out  = einsum('tij,btj->bti', w2, h1) + b2      # (B, D, O)

Sharding: the variable axis t (128) is split across 8 cores (16 vars each);
each core sees the full batch. Within a core, variables are processed in
pairs: a pair's two (64 x K) weight matrices are stacked/block-diagonalized
to fill the 128-wide tensor-engine array; activations live transposed
(feature-on-partition, batch-on-free).

Throughput structure:
- The PSUM->SBUF bias+leaky epilogues are the port-bound limiter (1
  elem/lane/cycle on ScalarE or VectorE); they are split between ScalarE
  (fused Prelu-with-bias) and VectorE (+GpSimd for the SBUF-side half).
- Emission is software-pipelined by batch chunk (L0 of chunk k, L1 of
  chunk k-1, L2 of chunk k-2) so the in-order TensorE queue never parks
  behind an epilogue, keeping the PE dense and HAM-warm.
- L1 runs as one 128x128 block-diagonal matmul per pair (L1_MODE="2x2"
  selects an alternative 2x2 tile_position packing; measured slower on
  HW because only diagonal quadrant pairs overlap).
- L2 packs ALL 8 pairs into one PSUM tile per chunk: pairs 4-7 matmul
  with zero-padded M=8 weights (start=True, landing at partition 32q+4),
  pairs 0-3 accumulate on top (M=4, start=False); a single VectorE
  bias-add + DMA evacuates the whole chunk. The host picks the 32 valid
  rows out of the wide (128, B) output.

Matmuls run in fp16 (1 cycle/row on the PE, fp32 accumulate in PSUM).
The kernel emits a (32, 8192) transposed output per core; the host
reassembles (8192, 128, 2).
"""

import numpy as np

import concourse.bass as bass
import concourse.mybir as mybir
import concourse.tile as tile
from concourse import bacc, bass_utils

F32 = mybir.dt.float32
DT = mybir.dt.float16
NPDT = np.float16

B = 8192  # batch
D = 128  # num variables (t)
H = 64  # hidden
O = 2  # output dim per variable
NCORES = 8
TPC = D // NCORES  # vars per core = 16
NPAIR = TPC // 2  # 8
CH = 512  # batch chunk (psum bank = 512 fp32)
NCHUNK = B // CH  # 16
ALPHA = 0.01  # leaky_relu slope

L1_MODE = "bd"  # "bd" = block-diag single MM per pair; "2x2" = tile_position

Prelu = mybir.ActivationFunctionType.Prelu
MULT = mybir.AluOpType.mult
MAX = mybir.AluOpType.max


def _epilogue_kind(c, p, layer):
    """Work split for the L0/L1 epilogues: 'act' (ScalarE, fused) or
    'dve' (VectorE bias-add + leaky on VectorE/GpSimd)."""
    idx = (c * NPAIR + p) * 2 + layer
    return "dve" if idx % 27 in (0, 3, 7, 10, 14, 17, 21, 24) else "act"


def _build_program():
    nc = bacc.Bacc(trn_type="TRN2")

    xt = nc.dram_tensor("xt", (D, B), DT, kind="ExternalInput")
    w0t = nc.dram_tensor("w0t", (D, NPAIR * 128), DT, kind="ExternalInput")
    w1bd = nc.dram_tensor("w1bd", (128, NPAIR * 128), DT, kind="ExternalInput")
    w2bd = nc.dram_tensor("w2bd", (128, NPAIR * 8), DT, kind="ExternalInput")
    b0c = nc.dram_tensor("b0c", (128, NPAIR), F32, kind="ExternalInput")
    b1c = nc.dram_tensor("b1c", (128, NPAIR), F32, kind="ExternalInput")
    # b2c: per-partition bias; partition 32q+r = pair q row r,
    # partition 32q+4+r = pair 4+q row r
    b2c = nc.dram_tensor("b2c", (128, 1), F32, kind="ExternalInput")
    # wide output: full 128-partition evac tiles; host picks valid rows
    otA = nc.dram_tensor("otA", (128, B), F32, kind="ExternalOutput")

    with tile.TileContext(nc) as tc:
        with (
            tc.tile_pool(name="wp", bufs=1) as wp,
            tc.tile_pool(name="hp", bufs=20) as hp,
            tc.tile_pool(name="op", bufs=4) as op,
            tc.tile_pool(name="z0p", bufs=4, space="PSUM") as z0p,
            tc.tile_pool(name="z1p", bufs=3, space="PSUM") as z1p,
            tc.tile_pool(name="z2p", bufs=1, space="PSUM") as z2p,
        ):
            xs = wp.tile([D, B], DT)
            xs_loaded = [False] * NCHUNK
            xs_loaded[0] = True
            w0s = wp.tile([D, NPAIR * 128], DT)
            w1s = wp.tile([128, NPAIR * 128], DT)
            w2s = wp.tile([128, NPAIR * 8], DT)
            b0s = wp.tile([128, NPAIR], F32)
            b1s = wp.tile([128, NPAIR], F32)
            b2s = wp.tile([128, 1], F32)
            nc.sync.dma_start(xs[:, 0:CH], xt[:, 0:CH])
            nc.sync.dma_start(w0s[:], w0t[:])
            nc.sync.dma_start(b0s[:], b0c[:])
            nc.sync.dma_start(w1s[:], w1bd[:])
            nc.sync.dma_start(b1s[:], b1c[:])
            nc.sync.dma_start(w2s[:], w2bd[:])
            nc.sync.dma_start(b2s[:], b2c[:])

            def leaky_epilogue(dst, z, bias_col, kind):
                """dst (fp16 SBUF) = leaky_relu(z + bias), z in PSUM."""
                if kind == "act":
                    nc.scalar.activation(
                        dst[:], z[:], Prelu, bias=bias_col, scale=1.0, alpha=ALPHA
                    )
                else:
                    y = hp.tile([128, CH], DT, tag="y", bufs=8)
                    nc.vector.tensor_scalar_add(y[:], z[:], bias_col)
                    eng = nc.gpsimd if kind == "dve_gps" else nc.vector
                    eng.scalar_tensor_tensor(dst[:], y[:], ALPHA, y[:], MULT, MAX)

            # PE warmup: dummy matmuls with no input-DMA dependency so the
            # HAM clock-gate reaches 8/8 while the input DMAs run.
            warm = wp.tile([128, CH], DT, name="warm")
            nc.gpsimd.memset(warm[:], 0.0)
            wps = z1p.tile([128, CH], F32, name="warmps", tag="z1")
            for _ in range(8):
                nc.tensor.matmul(wps[:], warm[:, 0:128], warm[:], start=True,
                                 stop=True)

            # per-chunk state carried across pipeline stages
            h0_tiles = [None] * NCHUNK  # list of 8 tiles per chunk
            h1_tiles = [None] * NCHUNK

            for k in range(NCHUNK + 2):
                # ---- stage A: L0 for chunk k ----
                if k < NCHUNK:
                    c = k
                    cs = bass.ts(c, CH)
                    if not xs_loaded[c]:
                        nc.sync.dma_start(xs[:, cs], xt[:, cs])
                        xs_loaded[c] = True
                    tiles = []
                    for p in range(NPAIR):
                        z0 = z0p.tile([128, CH], F32, tag="z0")
                        nc.tensor.matmul(
                            z0[:], w0s[:, bass.ts(p, 128)], xs[:, cs],
                            start=True, stop=True,
                        )
                        h0 = hp.tile([128, CH], DT, tag="h0", name=f"h0_{c}_{p}",
                                     bufs=20)
                        leaky_epilogue(h0, z0, b0s[:, p : p + 1],
                                       _epilogue_kind(c, p, 0))
                        tiles.append(h0)
                    h0_tiles[c] = tiles

                # ---- stage B: L1 for chunk k-1 (2x2 tile_position) ----
                if 1 <= k <= NCHUNK:
                    c = k - 1
                    tiles = []
                    z1s = []
                    if L1_MODE == "2x2":
                        for g in range(NPAIR // 2):
                            pA, pB = 2 * g, 2 * g + 1
                            h0A, h0B = h0_tiles[c][pA], h0_tiles[c][pB]
                            z1A = z1p.tile([128, CH], F32, tag="z1",
                                           name=f"z1A_{c}_{g}")
                            z1B = z1p.tile([128, CH], F32, tag="z1",
                                           name=f"z1B_{c}_{g}")
                            nc.tensor.matmul(
                                z1A[0:H, :], w1s[0:H, 128 * pA : 128 * pA + H],
                                h0A[0:H, :], start=True, stop=True,
                                tile_position=(0, 0),
                            )
                            nc.tensor.matmul(
                                z1A[H:128, :],
                                w1s[H:128, 128 * pA + H : 128 * pA + 128],
                                h0A[H:128, :], start=True, stop=True,
                                tile_position=(64, 64),
                            )
                            # pair B: outputs land var-swapped (odd var at 0:64)
                            nc.tensor.matmul(
                                z1B[H:128, :],
                                w1s[0:H, 128 * pB + H : 128 * pB + 128],
                                h0B[0:H, :], start=True, stop=True,
                                tile_position=(0, 64),
                            )
                            nc.tensor.matmul(
                                z1B[0:H, :], w1s[H:128, 128 * pB : 128 * pB + H],
                                h0B[H:128, :], start=True, stop=True,
                                tile_position=(64, 0),
                            )
                            z1s.extend([z1A, z1B])
                    else:
                        for p in range(NPAIR):
                            z1 = z1p.tile([128, CH], F32, tag="z1",
                                          name=f"z1_{c}_{p}")
                            nc.tensor.matmul(
                                z1[:], w1s[:, bass.ts(p, 128)],
                                h0_tiles[c][p][:], start=True, stop=True,
                            )
                            z1s.append(z1)
                    for p in range(NPAIR):
                        h1 = hp.tile([128, CH], DT, tag="h1", name=f"h1_{c}_{p}",
                                     bufs=20)
                        leaky_epilogue(h1, z1s[p], b1s[:, p : p + 1],
                                       _epilogue_kind(c, p, 1))
                        tiles.append(h1)
                    h1_tiles[c] = tiles

                # ---- stage C: L2 + evac + store for chunk k-2 ----
                if 2 <= k:
                    c = k - 2
                    cs = bass.ts(c, CH)
                    z2 = z2p.tile([128, CH], F32, name=f"z2_{c}", tag="z2")
                    for q in range(4):
                        # pair 4+q first: M=8 zero-padded weights write zeros
                        # into rows 32q..32q+4 (start=True clears), data into
                        # rows 32q+4..32q+8
                        pB = 4 + q
                        nc.tensor.matmul(
                            z2[32 * q : 32 * q + 8, :],
                            w2s[:, 8 * pB : 8 * pB + 8],
                            h1_tiles[c][pB][:],
                            start=True, stop=False,
                            tile_position=(0, 32 * q),
                        )
                        # pair q accumulates into rows 32q..32q+4
                        pA = q
                        nc.tensor.matmul(
                            z2[32 * q : 32 * q + 4, :],
                            w2s[:, 8 * pA : 8 * pA + 4],
                            h1_tiles[c][pA][:],
                            start=False, stop=True,
                            tile_position=(0, 32 * q),
                        )
                    ob = op.tile([128, CH], F32, tag="ob")
                    nc.vector.tensor_scalar_add(ob[:], z2[:], b2s[:, 0:1])
                    nc.sync.dma_start(otA[:, cs], ob[:])
                    h1_tiles[c] = None

    nc.finalize()
    return nc


_prog = None


def _get_program():
    global _prog
    if _prog is None:
        _prog = _build_program()
    return _prog


def _shard_inputs(x, w0, w1, w2, b0, b1, b2):
    """Host-side relayout + t-sharding. Returns list of 8 in_maps."""
    x = np.asarray(x, np.float32)
    w0 = np.array(w0, np.float32)  # copy: we zero the adjacency diagonal
    w1 = np.asarray(w1, np.float32)
    w2 = np.asarray(w2, np.float32)
    b0 = np.asarray(b0, np.float32)
    b1 = np.asarray(b1, np.float32)
    b2 = np.asarray(b2, np.float32)

    # adjacency mask: variable t cannot see itself -> w0[t, :, t] = 0
    ar = np.arange(D)
    w0[ar, :, ar] = 0.0

    xt = np.ascontiguousarray(x.T).astype(NPDT)  # (128, 8192)

    in_maps = []
    for c in range(NCORES):
        ts_ = slice(c * TPC, (c + 1) * TPC)
        w0c, w1c, w2c = w0[ts_], w1[ts_], w2[ts_]
        b0cc, b1cc, b2cc = b0[ts_], b1[ts_], b2[ts_]

        # w0t: (128 j, pair*128 + [ta's 64 i | tb's 64 i])
        w0T = w0c.transpose(0, 2, 1)  # (16, 128 j, 64 i)
        w0t_ = np.ascontiguousarray(
            w0T.reshape(NPAIR, 2, D, H).transpose(2, 0, 1, 3).reshape(D, NPAIR * 128)
        ).astype(NPDT)

        # w1bd: per-pair 128x128 blocks, K rows = h0 pair stack.
        # Even pairs: diagonal blocks; odd pairs: anti-diagonal (outputs come
        # out var-swapped from the 2x2 tile_position packing).
        bd1 = np.zeros((NPAIR, 128, 128), np.float32)
        for p in range(NPAIR):
            te, to = w1c[2 * p].T, w1c[2 * p + 1].T  # (in, out) each (64,64)
            if L1_MODE == "bd" or p % 2 == 0:
                bd1[p, 0:H, 0:H] = te
                bd1[p, H:128, H:128] = to
            else:
                bd1[p, 0:H, H:128] = te
                bd1[p, H:128, 0:H] = to
        w1bd_ = np.ascontiguousarray(
            bd1.transpose(1, 0, 2).reshape(128, NPAIR * 128)
        ).astype(NPDT)

        # b1c: per-partition bias for h1 tiles; odd pairs are var-swapped
        b1q = b1cc.reshape(NPAIR, 2, H)
        b1sw = b1q.copy()
        if L1_MODE == "2x2":
            b1sw[1::2] = b1q[1::2, ::-1]
        b1c_ = np.ascontiguousarray(b1sw.reshape(NPAIR, 128).T).astype(np.float32)

        # w2bd: (128 K, pair*4 + [ta o0, ta o1, tb o0, tb o1]); K rows follow
        # the h1 layout (odd pairs var-swapped), output cols stay canonical.
        bd2 = np.zeros((NPAIR, 128, 8), np.float32)
        for p in range(NPAIR):
            te, to = w2c[2 * p].T, w2c[2 * p + 1].T  # (64, 2) each
            off = 0 if p < 4 else 4  # B pairs sit 4 cols right (zero-padded)
            bd2[p, 0:H, off : off + 2] = te
            bd2[p, H:128, off + 2 : off + 4] = to
        w2bd_ = np.ascontiguousarray(
            bd2.transpose(1, 0, 2).reshape(128, NPAIR * 8)
        ).astype(NPDT)

        b0c_ = np.ascontiguousarray(b0cc.reshape(NPAIR, 128).T).astype(np.float32)
        # b2 bias layout for col-packed L2 psums: partition 32q+r of psum
        # `half` holds pair (4*half+q) row r (r = 2*two + o), canonical order
        b2q = b2cc.reshape(NPAIR, 4)
        b2c_ = np.zeros((128, 1), np.float32)
        for q in range(4):
            b2c_[32 * q : 32 * q + 4, 0] = b2q[q]
            b2c_[32 * q + 4 : 32 * q + 8, 0] = b2q[4 + q]

        in_maps.append(
            {
                "xt": xt,
                "w0t": w0t_,
                "w1bd": w1bd_,
                "w2bd": w2bd_,
                "b0c": b0c_,
                "b1c": b1c_,
                "b2c": b2c_,
            }
        )
    return in_maps


def _unshard_outputs(results):
    out = np.empty((B, D, O), np.float32)
    sel = np.array(
        [
            32 * p + r if p < 4 else 32 * (p - 4) + 4 + r
            for p in range(NPAIR)
            for r in range(4)
        ]
    )
    for c in range(NCORES):
        ot = results[c]["otA"][sel]  # (32, 8192): row = 4p + 2*two + o
        blk = ot.reshape(NPAIR, 2, O, B).transpose(3, 0, 1, 2).reshape(B, TPC, O)
        out[:, c * TPC : (c + 1) * TPC, :] = blk
    return out


def kernel(x, w0, w1, w2, b0, b1, b2):
    nc = _get_program()
    in_maps = _shard_inputs(x, w0, w1, w2, b0, b1, b2)
    res = bass_utils.run_bass_kernel_spmd(nc, in_maps, core_ids=list(range(NCORES)))
    return _unshard_outputs(res.results)
